# revision 6
# baseline (speedup 1.0000x reference)
"""ChebyKANLinear Trainium2 kernel.

Math: y[b,o] = (1/I) * sum_{i,d} T_d(c[b,i]) * W[i,o,d],  c = tanh(x)
with Chebyshev T_0=1, T_1=c, T_2=2c^2-1, T_3=4c^3-3c.
(The reference also clips c before arccos; the monomial recombination below
is exact on all of [-1,1], so the clip is irrelevant and dropped.)

Re-expressed in the monomial basis (exact linear recombination, folded into
the weights on the host):
    y = bias + c @ V1 + c^2 @ V2 + c^3 @ V3
    V1 = (W1 - 3*W3)/I, V2 = 2*W2/I, V3 = 4*W3/I, bias_o = sum_i (W0 - W2)[i,o]/I

Sharding: 2D — batch into 4 shards x output_dim into 2 shards across the 8
NeuronCores. Per core the matmuls are computed TRANSPOSED,
    yT[o, b] = sum_k  V_k[i, o].T @ (c^k)[i, b]
so each core runs only 6 matmuls of [K=128, M=128, N=512], and the bias
becomes a per-partition scalar fused into the PSUM->SBUF copy.

Everything rides in bf16 (rel-err budget is 2e-2; measured bf16 error is
~7e-3): halves the input DMA bytes, and a bf16 matmul is ONE PE pass where
fp32 needs two (LOW_HIGH split with doubled LDWEIGHTS). PSUM stays fp32.

Perf notes baked in from trace analysis (v2 trace, 19.7us):
- Per-DMA end-to-end latency is ~2.5us (issue ~0.6 + DGE fixed ~0.6 +
  engine delay ~0.65 + transfer + sem-propagation 0.9), and DMA
  completions serialize ~1.25us apart core-wide. So: few DMAs, spread
  over all three channels (sync HWDGE, scalar HWDGE, gpsimd SWDGE),
  issued in need-order.
- x-h0 rides sync (gates tanh -> whole chain), x-h1 rides the gpsimd
  SWDGE channel (v2 had it second on sync, landing 2.5us late and
  stalling the chain), V chunk 1 (first two weight blocks + bias) rides
  scalar first so the matmul chain can start, V chunk 2 follows.
- Warmup matmuls (fp32 on memset tiles) open the PE HAM clock-gate
  (1.2 -> 2.4 GHz) during the DMA phase; sized to end right as the real
  chain's operands land.
- Tail: bias pre-merge (acc_a + bias) split GpSimd/DVE so the final
  y = acc_b + tmp adds start the moment the last matmul retires; y is
  written bf16 as two half DMAs on the two HWDGE queues.
"""

from contextlib import ExitStack

import numpy as np
import ml_dtypes

import concourse.bass as bass
import concourse.tile as tile
from concourse import bacc, mybir
from concourse.bass_utils import run_bass_kernel_spmd

N_CORES = 8
B, I, O, D = 2048, 256, 256, 4
RB, SO = 4, 2  # batch shards x output shards
BL = B // RB  # 512 batch rows per core
OL = O // SO  # 128 output cols per core
F32 = mybir.dt.float32
BF16 = mybir.dt.bfloat16
NP_BF16 = ml_dtypes.bfloat16

# packed weight-column offsets, in matmul need-order:
# [V(0,0) | V(1,0) | bias | V(0,1) | V(2,0) | V(1,1) | V(2,1)]
_COL = {
    (0, 0): 0,
    (1, 0): OL,
    "bias": 2 * OL,
    (0, 1): 2 * OL + 1,
    (2, 0): 3 * OL + 1,
    (1, 1): 4 * OL + 1,
    (2, 1): 5 * OL + 1,
}
VB_W = 6 * OL + 1
VB_SPLIT = 2 * OL + 1  # chunk 1 = first two blocks + bias

_cache = {}


def _build_program():
    nc = bacc.Bacc("TRN2", target_bir_lowering=False, debug=False, num_devices=N_CORES)

    # [i_half, i_in_half, b_local]  (x slice pre-transposed + bf16-cast on host)
    xt_d = nc.dram_tensor("xt", [2, 128, BL], BF16, kind="ExternalInput")
    vb_d = nc.dram_tensor("vb", [128, VB_W], BF16, kind="ExternalInput")
    # transposed output [o_local, b_local], bf16 (host casts back to fp32)
    y_d = nc.dram_tensor("y", [OL, BL], BF16, kind="ExternalOutput")

    with tile.TileContext(nc) as tc, ExitStack() as ctx:
        pool = ctx.enter_context(tc.tile_pool(name="main", bufs=1))
        psum = ctx.enter_context(
            tc.tile_pool(name="psum", bufs=1, space=bass.MemorySpace.PSUM)
        )

        # PE warmup operands (DVE is idle this early; values are irrelevant)
        wu_w = pool.tile([128, 128], F32, tag="wu_w")
        nc.vector.memset(wu_w[:], 1.0)
        wu_r = pool.tile([128, 512], F32, tag="wu_r")
        nc.vector.memset(wu_r[:], 1.0)

        vb = pool.tile([128, VB_W], BF16, tag="vb")
        xt = {}
        for ih in range(2):
            xt[ih] = pool.tile([128, BL], BF16, tag=f"xt{ih}", name=f"xt{ih}")
        # need-order across the three DMA channels
        nc.sync.dma_start(xt[0][:], xt_d[0])
        nc.scalar.dma_start(vb[:, :VB_SPLIT], vb_d[:, :VB_SPLIT])
        nc.gpsimd.dma_start(xt[1][:], xt_d[1])
        nc.scalar.dma_start(vb[:, VB_SPLIT:], vb_d[:, VB_SPLIT:])

        # Warmup: fp32 N=512 (two LOW/HIGH passes) + fp32 N=128, ending
        # right as tanh(h0) + V chunk 1 land.
        wu_acc = psum.tile([128, 512], F32, tag="wu_acc")
        nc.tensor.matmul(wu_acc[:], wu_w[:], wu_r[:], start=True, stop=True)
        nc.tensor.matmul(
            wu_acc[:, :128], wu_w[:], wu_r[:, :128], start=True, stop=True
        )

        # basis: c = tanh(xT) on ACT, c^2/c^3 on DVE (all bf16)
        basis = {}
        hb = BL // 2
        c0 = pool.tile([128, BL], BF16, tag="c0")
        nc.scalar.activation(c0[:], xt[0][:], mybir.ActivationFunctionType.Tanh)
        basis[(0, 0)] = c0
        c1 = pool.tile([128, BL], BF16, tag="c1")
        nc.scalar.activation(c1[:], xt[1][:], mybir.ActivationFunctionType.Tanh)
        basis[(0, 1)] = c1
        for ih in range(2):
            c2 = pool.tile([128, BL], BF16, tag=f"c2{ih}")
            nc.vector.tensor_mul(c2[:], basis[(0, ih)][:], basis[(0, ih)][:])
            basis[(1, ih)] = c2
        for ih in range(2):
            c3 = pool.tile([128, BL], BF16, tag=f"c3{ih}")
            nc.vector.tensor_mul(c3[:], basis[(1, ih)][:], basis[(0, ih)][:])
            basis[(2, ih)] = c3

        # tensor_scalar needs an fp32 scalar: GpSimd up-converts the bf16
        # bias column once V chunk 1 lands, long before the tail needs it.
        bias_col = pool.tile([128, 1], F32, tag="bias_f32")
        nc.gpsimd.tensor_copy(bias_col[:], vb[:, _COL["bias"] : _COL["bias"] + 1])

        # yT[o, b] accumulation: 6 matmuls alternating between TWO PSUM
        # banks (ih=0 -> acc_a, ih=1 -> acc_b) so consecutive accumulating
        # passes don't serialize on one bank.
        acc_a = psum.tile([128, BL], F32, tag="acc_a")
        acc_b = psum.tile([128, BL], F32, tag="acc_b")
        accs = {0: acc_a, 1: acc_b}
        # ordered by operand readiness
        mm_order = [(0, 0), (1, 0), (0, 1), (2, 0), (1, 1), (2, 1)]
        for d, ih in mm_order:
            col = _COL[(d, ih)]
            nc.tensor.matmul(
                accs[ih][:OL, :],
                vb[:, col : col + OL],
                basis[(d, ih)][:],
                start=(d == 0),
                stop=(d == 2),
            )

        # DVE can read only ONE PSUM operand per op: pre-merge acc_a + bias
        # into SBUF (overlaps the final acc_b matmuls; GpSimd cannot touch
        # PSUM on TRN2, but ACT can and is idle after the tanhs — split the
        # pre-merge ACT/DVE so DVE stays free for the cube muls), then
        # y = acc_b + tmp.
        tmp_sb = pool.tile([OL, BL], BF16, tag="tmp_sb")
        y_sb = pool.tile([OL, BL], BF16, tag="y_sb")
        nc.scalar.activation(
            tmp_sb[:, :hb],
            acc_a[:OL, :hb],
            mybir.ActivationFunctionType.Identity,
            bias=bias_col[:],
        )
        nc.vector.tensor_scalar_add(tmp_sb[:, hb:], acc_a[:OL, hb:], bias_col[:])
        for k in range(2):
            s = slice(k * hb, (k + 1) * hb)
            nc.vector.tensor_tensor(
                y_sb[:, s], acc_b[:OL, s], tmp_sb[:, s], mybir.AluOpType.add
            )
            (nc.sync if k == 0 else nc.scalar).dma_start(y_d[:, s], y_sb[:, s])

    nc.compile()
    return nc


def _get_program():
    if "nc" not in _cache:
        _cache["nc"] = _build_program()
    return _cache["nc"]


def _make_in_maps(x, cheby_coeffs):
    x = np.ascontiguousarray(x, dtype=np.float32)
    W = np.ascontiguousarray(cheby_coeffs, dtype=np.float32)
    assert x.shape == (B, I) and W.shape == (I, O, D)

    inv_i = np.float32(1.0 / I)
    V = np.stack(
        [
            W[:, :, 1] - 3.0 * W[:, :, 3],
            2.0 * W[:, :, 2],
            4.0 * W[:, :, 3],
        ]
    ).astype(np.float32) * inv_i  # [3, I, O]
    bias_full = (W[:, :, 0] - W[:, :, 2]).sum(axis=0, dtype=np.float32) * inv_i  # [O]

    xt_shards = []
    for rb in range(RB):
        xs = x[rb * BL : (rb + 1) * BL, :]  # [BL, I]
        xt_shards.append(
            np.ascontiguousarray(xs.T.astype(NP_BF16)).reshape(2, 128, BL)
        )
    vb_shards = []
    for so in range(SO):
        vb = np.empty((128, VB_W), dtype=NP_BF16)
        osl = slice(so * OL, (so + 1) * OL)
        for key, col in _COL.items():
            if key == "bias":
                continue
            d, ih = key
            vb[:, col : col + OL] = V[d, ih * 128 : (ih + 1) * 128, osl].astype(
                NP_BF16
            )
        vb[:, _COL["bias"]] = bias_full[osl].astype(NP_BF16)
        vb_shards.append(vb)
    in_maps = []
    for c_id in range(N_CORES):
        rb, so = divmod(c_id, SO)
        in_maps.append({"xt": xt_shards[rb], "vb": vb_shards[so]})
    return in_maps


def kernel(x, cheby_coeffs):
    nc = _get_program()
    in_maps = _make_in_maps(x, cheby_coeffs)
    res = run_bass_kernel_spmd(nc, in_maps, list(range(N_CORES)))
    y = np.empty((B, O), dtype=np.float32)
    for c_id in range(N_CORES):
        rb, so = divmod(c_id, SO)
        y[rb * BL : (rb + 1) * BL, so * OL : (so + 1) * OL] = (
            res.results[c_id]["y"].astype(np.float32).T
        )
    return y
